# revision 55
# baseline (speedup 1.0000x reference)
"""CIM signed-magnitude linear kernel v4.4 — fp16 transport, fp16 stores.

Math identity (exact): y = (x_q @ w_q.T) * scale_x * scale_w.T + bias with
x_q = round(x / (max|x|/127 + eps)) per token, w_q likewise per out-channel.
(The reference's bit-serial/ADC path is numerically identical: per-chunk
partial sums are <= 64 so the [0,1023] ADC clamp never binds, and rounding
of integers is exact; int products accumulate exactly in f32 PSUM.)

Design (vs the f32 v2 baseline):
  * fp16 HBM transport for x / w / bias (host cast) — input DMA halves to
    ~2MB/core; fp16 output stores halve store traffic.  fp16 cast error
    ~2^-11/elem, small against the 2e-2 rel-err budget (measured 2.8e-3).
  * One-pass quantize: q_fp16 = (x * xinv) + 1536.0; the single f32->fp16
    rounding lands on the integer grid (fp16 ulp is exactly 1.0 on
    [1024, 2048)); the -1536 de-bias rides the mandatory PSUM->SBUF evict.
  * Per-operand cost is pass-count minimal: absmax reduce (DVE-only op) +
    quantize + transpose-evict.  The DVE reduce ladder (8x ~1.2us, no HW
    fast mode for reduce) is the critical resource; everything else is
    kept off DVE: quants on Pool/ACT, evicts on ACT in full-PSUM-bank
    [128,1024] ops, scale prep on Pool.
  * 2-hop out-pass: otmp_f32 = DVE TT(ps * xmax-broadcast) straight from
    PSUM (f32 intermediate — ps*xmax overflows fp16), then y = Pool
    TS(otmp * wmax/16129 + bias) using gpsimd's fast mult-AP+add-AP form
    (single-op add-AP falls onto a ~10x slower software path).
  * PE clock (HAM) management: 32 junk N=128 matmuls into ps[3] (cleared
    by mm_group(3)'s start=True) gated on the first x tile, plus a 12-MM
    bridge burst before the w chains — transposes never count as HAM
    activity, so only real MMs keep the PE at 2.4GHz.
  * bcx16 = broadcast of per-token xmax via PE column transposes + ones
    matmul.  Beware PSUM bank collisions: a PE write and an engine read to
    different addresses of the SAME bank is a HW fault, so each staging
    row/broadcast gets its own bank region ordering via overlapping APs.

Sharding: 8 cores = 4 token-shards x 2 out-feature shards, no collectives.
"""

import os

os.environ.setdefault("JAX_PLATFORMS", "cpu")

import numpy as np

B, S, IN_F, OUT_F = 2, 1024, 1024, 1024
T = B * S
M_SHARDS, N_SHARDS = 4, 2
TC = T // M_SHARDS             # 512 tokens per core
OC = OUT_F // N_SHARDS         # 512 out-features per core
NT = TC // 128                 # 4 token tiles
NO = OC // 128                 # 4 out-feature tiles
KB = IN_F // 128               # 8 contraction blocks
WROW = IN_F + 1                # weight row with bias appended

EPS = 1e-8
INV127 = 1.0 / 127.0
INV16129 = 1.0 / 16129.0
# fp16 has 10 mantissa bits: on [1024,2048) the ulp is exactly 1.0, so the
# single f32->fp16 output rounding of (q + 1536) lands on the integer grid.
MAGIC = 1536.0
N_WARMUP = 0

_CACHE = {}


def _build_nc():
    import concourse.bass as bass
    import concourse.mybir as mybir
    import concourse.tile as tile
    from concourse.masks import make_identity

    F32 = mybir.dt.float32
    F16 = mybir.dt.float16
    ALU = mybir.AluOpType
    ACTF = mybir.ActivationFunctionType
    AX = mybir.AxisListType

    nc = bass.Bass("TRN2", target_bir_lowering=False, debug=False)

    x_d = nc.dram_tensor("x", [TC, IN_F], F16, kind="ExternalInput").ap()
    wb_d = nc.dram_tensor("wb", [OC, WROW], F16, kind="ExternalInput").ap()
    out_d = nc.dram_tensor("out", [OC, TC], F16, kind="ExternalOutput").ap()

    x3 = x_d.rearrange("(q p) i -> p q i", p=128)     # [128, NT, IN_F]
    w3 = wb_d.rearrange("(r p) i -> p r i", p=128)    # [128, NO, WROW]

    with tile.TileContext(nc) as tc:
        with (
            tc.tile_pool(name="raw", bufs=1) as raw,
            tc.tile_pool(name="qb", bufs=1) as qb,
            tc.tile_pool(name="small", bufs=1) as small,
            tc.tile_pool(name="ob", bufs=2) as obp,
            tc.tile_pool(name="mmps", bufs=4, space="PSUM") as mmps,
            tc.tile_pool(name="stps", bufs=2, space="PSUM") as stps,
            tc.tile_pool(name="auxps", bufs=1, space="PSUM") as auxps,
        ):
            x_sb = raw.tile([128, NT, IN_F], F16, tag="x_sb")
            w_sb = raw.tile([128, NO, WROW], F16, tag="w_sb")
            xq = qb.tile([128, NT, IN_F], F16, tag="xq")      # values +1536
            wq = qb.tile([128, NO, IN_F], F16, tag="wq")      # values +1536
            xqT = qb.tile([128, KB, TC], F16, tag="xqT")      # true ints
            wqT = qb.tile([128, KB, OC], F16, tag="wqT")      # true ints
            bcx16 = qb.tile([128, TC], F16, tag="bcx16")      # xmax broadcast
            row_sb = qb.tile([1, TC], F16, tag="row_sb")
            xinvrow = qb.tile([1, TC], F16, tag="xinvrow")    # (1/xmax)/127 row
            pbrow = qb.tile([1, OC], F16, tag="pbrow")        # 127*b*winv row
            ones1 = qb.tile([1, 128], F16, tag="ones1")
            ident_f16 = qb.tile([128, 128], F16, tag="ident_f16")

            # fp16 reduce outputs: max|fp16| is exactly representable in
            # fp16, and an all-2-byte instruction lets the DVE engage its
            # 2x/4x perf modes.
            xmax = small.tile([128, NT], F16, tag="xmax")
            xden = small.tile([128, NT], F32, tag="xden")
            xinv = small.tile([128, NT], F32, tag="xinv")
            xinv16 = small.tile([128, NT], F16, tag="xinv16")  # xinv/127
            pb16 = small.tile([128, NO], F16, tag="pb16")      # 127*b*winv
            wmax = small.tile([128, NO], F16, tag="wmax")
            wmaxs = small.tile([128, NO], F32, tag="wmaxs")   # wmax/16129, f32
            wmaxh = small.tile([128, 2], F16, tag="wmaxh")    # w3 half-maxes
            wden = small.tile([128, NO], F32, tag="wden")
            winv = small.tile([128, NO], F32, tag="winv")
            bias2 = small.tile([128, NO], F32, tag="bias2")

            ps = [mmps.tile([128, TC], F32, tag="ps", name=f"ps{m}")
                  for m in range(NO)]
            st = [stps.tile([128, 1024], F16, tag="st", name=f"st{i}")
                  for i in range(2)]
            rowps = auxps.tile([128, 1024], F16, tag="rowps")  # row staging
            bcps = auxps.tile([128, TC], F32, tag="bcps")     # bcx broadcast

            # ---- constants ----
            nc.gpsimd.memset(ones1, 1.0)
            make_identity(nc, ident_f16)
            # preload the ACT activation table during the load phase so the
            # first real ACTIVATE doesn't eat the 1.3us ACT_TABLE_LOAD
            nc.scalar.activation(out=row_sb[0:1, 0:128], in_=ones1,
                                 func=ACTF.Copy, scale=1.0, bias=0.0)

            # ---- loads: x tile pairs first, w tiles after, w3 split ----
            for q in range(0, NT, 2):
                nc.sync.dma_start(out=x_sb[:, q:q + 2], in_=x3[:, q:q + 2])
            for r in range(NO - 1):
                nc.sync.dma_start(out=w_sb[:, r:r + 1], in_=w3[:, r:r + 1])
            nc.sync.dma_start(out=w_sb[:, 3, 0:512], in_=w3[:, 3, 0:512])
            nc.sync.dma_start(out=w_sb[:, 3, 512:WROW], in_=w3[:, 3, 512:WROW])

            # ---- PE warmup: HAM counts only real matmuls, so stream junk
            # N=128 MMs (into ps[3], cleared later by mm_group(3)'s
            # start=True) as soon as the first x tile lands — flips the PE
            # clock gate to 2.4GHz before the real transposes and MMs.
            for i in range(32):
                nc.tensor.matmul(ps[3][:, 0:128], lhsT=ident_f16,
                                 rhs=x_sb[:, 0, 0:128], start=True, stop=True)

            def x_red(q):
                nc.vector.tensor_reduce(
                    out=xmax[:, q:q + 1], in_=x_sb[:, q, :], axis=AX.X,
                    op=ALU.max, apply_absolute_value=True)
                nc.vector.tensor_scalar(
                    out=xden[:, q:q + 1], in0=xmax[:, q:q + 1],
                    scalar1=INV127, scalar2=EPS, op0=ALU.mult, op1=ALU.add)
                nc.vector.reciprocal(out=xinv[:, q:q + 1], in_=xden[:, q:q + 1])

            def x_quant_act(q, c0, c1):
                nc.scalar.activation(
                    out=xq[:, q, c0:c1], in_=x_sb[:, q, c0:c1], func=ACTF.Copy,
                    scale=xinv[:, q:q + 1], bias=MAGIC)

            def x_quant_pool(q, c0, c1):
                nc.gpsimd.tensor_scalar(
                    out=xq[:, q, c0:c1], in0=x_sb[:, q, c0:c1],
                    scalar1=xinv[:, q:q + 1], scalar2=MAGIC,
                    op0=ALU.mult, op1=ALU.add)

            def x_T(q):
                # all 8 contraction blocks of token-tile q into one st bank
                bank = st[q % 2]
                for k in range(KB):
                    nc.tensor.transpose(
                        bank[:, k * 128:(k + 1) * 128],
                        xq[:, q, k * 128:(k + 1) * 128], ident_f16)
                return bank

            def x_ev(q, bank, eng):
                # full-bank evict: st bank -> xqT[:, :, q-block], de-bias
                out_ap = xqT[:, 0:KB, q * 128:(q + 1) * 128]
                in_ap = bank.rearrange("p (k c) -> p k c", c=128)
                if eng == "act":
                    nc.scalar.activation(out=out_ap, in_=in_ap,
                                         func=ACTF.Copy, scale=1.0, bias=-MAGIC)
                else:
                    nc.vector.tensor_scalar(out=out_ap, in0=in_ap,
                                            scalar1=-MAGIC, scalar2=None,
                                            op0=ALU.add)

            def w_red(r):
                nc.vector.tensor_reduce(
                    out=wmax[:, r:r + 1], in_=w_sb[:, r, 0:IN_F], axis=AX.X,
                    op=ALU.max, apply_absolute_value=True)

            def w_den(r):
                nc.vector.tensor_scalar(
                    out=wden[:, r:r + 1], in0=wmax[:, r:r + 1],
                    scalar1=INV127, scalar2=EPS, op0=ALU.mult, op1=ALU.add)
                nc.vector.reciprocal(out=winv[:, r:r + 1], in_=wden[:, r:r + 1])
                nc.gpsimd.tensor_scalar(
                    out=wmaxs[:, r:r + 1], in0=wmax[:, r:r + 1],
                    scalar1=INV16129, scalar2=0.0,
                    op0=ALU.mult, op1=ALU.add)

            def pb_chain(r):
                # pbrow[o] = 127*b[o]*winv[o] as a row, staged via PE transpose
                nc.vector.tensor_tensor(
                    out=pb16[:, r:r + 1], in0=bias2[:, r:r + 1],
                    in1=winv[:, r:r + 1], op=ALU.mult)
                nc.tensor.transpose(
                    rowps[0:1, r * 128:(r + 1) * 128], pb16[:, r:r + 1],
                    ident_f16)
                nc.scalar.activation(
                    out=pbrow[0:1, r * 128:(r + 1) * 128],
                    in_=rowps[0:1, r * 128:(r + 1) * 128], func=ACTF.Copy,
                    scale=1.0, bias=0.0)

            def w_quant(r, c0, c1, eng):
                if eng == "act":
                    nc.scalar.activation(
                        out=wq[:, r, c0:c1], in_=w_sb[:, r, c0:c1],
                        func=ACTF.Copy, scale=winv[:, r:r + 1], bias=MAGIC)
                else:
                    nc.gpsimd.tensor_scalar(
                        out=wq[:, r, c0:c1], in0=w_sb[:, r, c0:c1],
                        scalar1=winv[:, r:r + 1], scalar2=MAGIC,
                        op0=ALU.mult, op1=ALU.add)

            def w_T(r):
                bank = st[r % 2]
                for k in range(KB):
                    nc.tensor.transpose(
                        bank[:, k * 128:(k + 1) * 128],
                        wq[:, r, k * 128:(k + 1) * 128], ident_f16)
                return bank

            def w_ev(r, bank, eng):
                out_ap = wqT[:, 0:KB, r * 128:(r + 1) * 128]
                in_ap = bank.rearrange("p (k c) -> p k c", c=128)
                if eng == "act":
                    nc.scalar.activation(out=out_ap, in_=in_ap,
                                         func=ACTF.Copy, scale=1.0, bias=-MAGIC)
                else:
                    nc.vector.tensor_scalar(out=out_ap, in0=in_ap,
                                            scalar1=-MAGIC, scalar2=None,
                                            op0=ALU.add)

            def mm_group(m):
                for k in range(KB):
                    nc.tensor.matmul(
                        ps[m], lhsT=wqT[:, k, m * 128:(m + 1) * 128],
                        rhs=xqT[:, k, :], start=(k == 0), stop=False)
                # bias as a rank-1 K=1 accumulation: contributes
                # 127*b*winv[o] * (xinv[t]/127) = b/(sw*sx) to ps, which the
                # out-pass scaling turns back into exactly +b.
                nc.tensor.matmul(
                    ps[m], lhsT=pbrow[0:1, m * 128:(m + 1) * 128],
                    rhs=xinvrow, start=False, stop=True)

            def out_evict(m):
                # u = ps * wmax/16129 (per-partition), fp16 out; PSUM on ACT
                u = obp.tile([128, TC], F16, tag="u", name=f"u{m}")
                nc.scalar.activation(out=u, in_=ps[m], func=ACTF.Copy,
                                     scale=wmaxs[:, m:m + 1], bias=0.0)
                # y = u * xmax[t]-broadcast (fp16 all-SBUF: DVE fast mode)
                osb = obp.tile([128, TC], F16, tag="osb", name=f"osb{m}")
                nc.vector.tensor_tensor(out=osb, in0=u, in1=bcx16,
                                        op=ALU.mult)
                nc.sync.dma_start(out=out_d[m * 128:(m + 1) * 128, :], in_=osb)

            # ================= pipelined emission (approx time order) ======
            # --- x chains ---
            x_red(0)
            x_quant_act(0, 0, 512)
            x_quant_pool(0, 512, IN_F)
            x_T(0)
            x_red(1)
            x_quant_pool(1, 0, IN_F)
            x_ev(0, st[0], "act")
            x_T(1)
            x_red(2)
            x_quant_act(2, 0, 512)
            x_quant_pool(2, 512, IN_F)
            x_ev(1, st[1], "act")
            x_T(2)
            x_red(3)
            x_quant_pool(3, 0, IN_F)
            x_ev(2, st[0], "act")
            x_T(3)
            # token-scale row: 4 single-column PE transposes into rowps
            for q in range(NT):
                nc.tensor.transpose(
                    rowps[0:1, q * 128:(q + 1) * 128], xmax[:, q:q + 1],
                    ident_f16)
            nc.scalar.activation(
                out=row_sb, in_=rowps[0:1, 0:512], func=ACTF.Copy,
                scale=1.0, bias=0.0)
            nc.tensor.matmul(bcps, lhsT=ones1, rhs=row_sb,
                             start=True, stop=True)
            # 1/(127*xmax) row for the bias rank-1 accumulation
            nc.vector.tensor_copy(out=xinv16, in_=xinv)
            for q in range(NT):
                nc.tensor.transpose(
                    rowps[0:1, 512 + q * 128:512 + (q + 1) * 128],
                    xinv16[:, q:q + 1], ident_f16)
            nc.scalar.activation(
                out=xinvrow, in_=rowps[0:1, 512:1024], func=ACTF.Copy,
                scale=INV127, bias=0.0)
            x_ev(3, st[1], "act")
            # bridge the PE idle gap before the w chains so HAM stays warm
            for i in range(12):
                nc.tensor.matmul(ps[3][:, 0:128], lhsT=ident_f16,
                                 rhs=x_sb[:, 0, 0:128], start=True, stop=True)

            # --- w chains ---
            w_red(0)
            w_den(0)
            w_quant(0, 0, IN_F, "pool")
            nc.gpsimd.tensor_copy(out=bias2[:, 0:1], in_=w_sb[:, 0, IN_F:WROW])
            w_T(0)
            w_red(1)
            w_den(1)
            w_ev(0, st[0], "act")
            w_quant(1, 0, IN_F, "pool")
            nc.gpsimd.tensor_copy(out=bias2[:, 1:2], in_=w_sb[:, 1, IN_F:WROW])
            w_T(1)
            nc.vector.tensor_copy(out=bcx16, in_=bcps)
            mm_group(0)
            w_red(2)
            w_den(2)
            w_ev(1, st[1], "act")
            w_quant(2, 0, IN_F, "pool")
            nc.gpsimd.tensor_copy(out=bias2[:, 2:3], in_=w_sb[:, 2, IN_F:WROW])
            w_T(2)
            mm_group(1)
            out_evict(0)
            # w3: reduce halves on DVE, quant ACT || Pool
            nc.vector.tensor_reduce(
                out=wmaxh[:, 0:1], in_=w_sb[:, 3, 0:512], axis=AX.X,
                op=ALU.max, apply_absolute_value=True)
            nc.vector.tensor_reduce(
                out=wmaxh[:, 1:2], in_=w_sb[:, 3, 512:IN_F], axis=AX.X,
                op=ALU.max, apply_absolute_value=True)
            nc.vector.tensor_tensor(
                out=wmax[:, 3:4], in0=wmaxh[:, 0:1], in1=wmaxh[:, 1:2],
                op=ALU.max)
            w_den(3)
            w_ev(2, st[0], "act")
            w_quant(3, 0, 512, "act")
            w_quant(3, 512, IN_F, "pool")
            nc.gpsimd.tensor_copy(out=bias2[:, 3:4], in_=w_sb[:, 3, IN_F:WROW])
            w_T(3)
            mm_group(2)
            out_evict(1)
            w_ev(3, st[1], "act")
            mm_group(3)
            out_evict(2)
            out_evict(3)

    _split_multiwaits(nc)
    return nc


def _split_multiwaits(nc):
    """Hoist all but one wait of any multi-wait instruction into standalone
    EventSemaphore instructions (the ISA carries one wait per instruction)."""
    import concourse.mybir as mybir

    fn = nc.m.functions[0]
    ctr = [0]
    for blk in fn.blocks:
        insts = list(blk.instructions)
        changed = False
        out = []
        for inst in insts:
            si = inst.sync_info
            waits = list(si.on_wait or []) if si is not None else []
            if len(waits) > 1:
                for w in waits[:-1]:
                    ctr[0] += 1
                    es = mybir.InstEventSemaphore(
                        name=f"I-eswait-{ctr[0]}", engine=inst.engine,
                        ins=[], outs=[],
                    )
                    es.sync_info = mybir.SyncInfo(on_wait=[w], on_update=[])
                    out.append(es)
                    nc.register_instruction(es)
                inst.sync_info = mybir.SyncInfo(
                    on_wait=[waits[-1]], on_update=list(si.on_update or []),
                )
                changed = True
            out.append(inst)
        if changed:
            blk.instructions = out


def get_nc():
    if "nc" not in _CACHE:
        _CACHE["nc"] = _build_nc()
    return _CACHE["nc"]


def make_in_maps(x, weight, bias):
    xf = np.asarray(x, dtype=np.float32).reshape(T, IN_F).astype(np.float16)
    w = np.asarray(weight, dtype=np.float32).astype(np.float16)
    b = np.asarray(bias, dtype=np.float32).astype(np.float16)
    wb = np.concatenate([w, b[:, None]], axis=1)
    in_maps = []
    for c in range(M_SHARDS * N_SHARDS):
        im, jn = divmod(c, N_SHARDS)
        in_maps.append({
            "x": np.ascontiguousarray(xf[im * TC:(im + 1) * TC]),
            "wb": np.ascontiguousarray(wb[jn * OC:(jn + 1) * OC]),
        })
    return in_maps


def assemble(results):
    y = np.empty((T, OUT_F), dtype=np.float32)
    for c in range(M_SHARDS * N_SHARDS):
        im, jn = divmod(c, N_SHARDS)
        y[im * TC:(im + 1) * TC, jn * OC:(jn + 1) * OC] = \
            results[c]["out"].T.astype(np.float32)
    return y.reshape(B, S, OUT_F)


def run(x, weight, bias, **spmd_kwargs):
    from concourse.bass_utils import run_bass_kernel_spmd

    nc = get_nc()
    in_maps = make_in_maps(x, weight, bias)
    res = run_bass_kernel_spmd(nc, in_maps, core_ids=list(range(8)), **spmd_kwargs)
    return assemble(res.results), res


def kernel(x, weight, bias):
    y, _ = run(x, weight, bias)
    return y


# revision 56
# speedup vs baseline: 1.0011x; 1.0011x over previous
"""CIM signed-magnitude linear kernel v4.4 — fp16 transport, fp16 stores.

Math identity (exact): y = (x_q @ w_q.T) * scale_x * scale_w.T + bias with
x_q = round(x / (max|x|/127 + eps)) per token, w_q likewise per out-channel.
(The reference's bit-serial/ADC path is numerically identical: per-chunk
partial sums are <= 64 so the [0,1023] ADC clamp never binds, and rounding
of integers is exact; int products accumulate exactly in f32 PSUM.)

Design (vs the f32 v2 baseline):
  * fp16 HBM transport for x / w / bias (host cast) — input DMA halves to
    ~2MB/core; fp16 output stores halve store traffic.  fp16 cast error
    ~2^-11/elem, small against the 2e-2 rel-err budget (measured 2.8e-3).
  * One-pass quantize: q_fp16 = (x * xinv) + 1536.0; the single f32->fp16
    rounding lands on the integer grid (fp16 ulp is exactly 1.0 on
    [1024, 2048)); the -1536 de-bias rides the mandatory PSUM->SBUF evict.
  * Per-operand cost is pass-count minimal: absmax reduce (DVE-only op) +
    quantize + transpose-evict.  The DVE reduce ladder (8x ~1.2us, no HW
    fast mode for reduce) is the critical resource; everything else is
    kept off DVE: quants on Pool/ACT, evicts on ACT in full-PSUM-bank
    [128,1024] ops, scale prep on Pool.
  * 2-hop out-pass: otmp_f32 = DVE TT(ps * xmax-broadcast) straight from
    PSUM (f32 intermediate — ps*xmax overflows fp16), then y = Pool
    TS(otmp * wmax/16129 + bias) using gpsimd's fast mult-AP+add-AP form
    (single-op add-AP falls onto a ~10x slower software path).
  * PE clock (HAM) management: 32 junk N=128 matmuls into ps[3] (cleared
    by mm_group(3)'s start=True) gated on the first x tile, plus a 12-MM
    bridge burst before the w chains — transposes never count as HAM
    activity, so only real MMs keep the PE at 2.4GHz.
  * bcx16 = broadcast of per-token xmax via PE column transposes + ones
    matmul.  Beware PSUM bank collisions: a PE write and an engine read to
    different addresses of the SAME bank is a HW fault, so each staging
    row/broadcast gets its own bank region ordering via overlapping APs.

Sharding: 8 cores = 4 token-shards x 2 out-feature shards, no collectives.
"""

import os

os.environ.setdefault("JAX_PLATFORMS", "cpu")

import numpy as np

B, S, IN_F, OUT_F = 2, 1024, 1024, 1024
T = B * S
M_SHARDS, N_SHARDS = 4, 2
TC = T // M_SHARDS             # 512 tokens per core
OC = OUT_F // N_SHARDS         # 512 out-features per core
NT = TC // 128                 # 4 token tiles
NO = OC // 128                 # 4 out-feature tiles
KB = IN_F // 128               # 8 contraction blocks
WROW = IN_F + 1                # weight row with bias appended

EPS = 1e-8
INV127 = 1.0 / 127.0
INV16129 = 1.0 / 16129.0
# fp16 has 10 mantissa bits: on [1024,2048) the ulp is exactly 1.0, so the
# single f32->fp16 output rounding of (q + 1536) lands on the integer grid.
MAGIC = 1536.0
N_WARMUP = 0

_CACHE = {}


def _build_nc():
    import concourse.bass as bass
    import concourse.mybir as mybir
    import concourse.tile as tile
    from concourse.masks import make_identity

    F32 = mybir.dt.float32
    F16 = mybir.dt.float16
    ALU = mybir.AluOpType
    ACTF = mybir.ActivationFunctionType
    AX = mybir.AxisListType

    nc = bass.Bass("TRN2", target_bir_lowering=False, debug=False)

    x_d = nc.dram_tensor("x", [TC, IN_F], F16, kind="ExternalInput").ap()
    wb_d = nc.dram_tensor("wb", [OC, WROW], F16, kind="ExternalInput").ap()
    out_d = nc.dram_tensor("out", [OC, TC], F16, kind="ExternalOutput").ap()

    x3 = x_d.rearrange("(q p) i -> p q i", p=128)     # [128, NT, IN_F]
    w3 = wb_d.rearrange("(r p) i -> p r i", p=128)    # [128, NO, WROW]

    with tile.TileContext(nc) as tc:
        with (
            tc.tile_pool(name="raw", bufs=1) as raw,
            tc.tile_pool(name="qb", bufs=1) as qb,
            tc.tile_pool(name="small", bufs=1) as small,
            tc.tile_pool(name="ob", bufs=2) as obp,
            tc.tile_pool(name="mmps", bufs=4, space="PSUM") as mmps,
            tc.tile_pool(name="stps", bufs=2, space="PSUM") as stps,
            tc.tile_pool(name="auxps", bufs=1, space="PSUM") as auxps,
        ):
            x_sb = raw.tile([128, NT, IN_F], F16, tag="x_sb")
            w_sb = raw.tile([128, NO, WROW], F16, tag="w_sb")
            xq = qb.tile([128, NT, IN_F], F16, tag="xq")      # values +1536
            wq = qb.tile([128, NO, IN_F], F16, tag="wq")      # values +1536
            xqT = qb.tile([128, KB, TC], F16, tag="xqT")      # true ints
            wqT = qb.tile([128, KB, OC], F16, tag="wqT")      # true ints
            bcx16 = qb.tile([128, TC], F16, tag="bcx16")      # xmax broadcast
            row_sb = qb.tile([1, TC], F16, tag="row_sb")
            xinvrow = qb.tile([1, TC], F16, tag="xinvrow")    # (1/xmax)/127 row
            pbrow = qb.tile([1, OC], F16, tag="pbrow")        # 127*b*winv row
            ones1 = qb.tile([1, 128], F16, tag="ones1")
            ident_f16 = qb.tile([128, 128], F16, tag="ident_f16")

            # fp16 reduce outputs: max|fp16| is exactly representable in
            # fp16, and an all-2-byte instruction lets the DVE engage its
            # 2x/4x perf modes.
            xmax = small.tile([128, NT], F16, tag="xmax")
            xden = small.tile([128, NT], F32, tag="xden")
            xinv = small.tile([128, NT], F32, tag="xinv")
            xinv16 = small.tile([128, NT], F16, tag="xinv16")  # xinv/127
            pb16 = small.tile([128, NO], F16, tag="pb16")      # 127*b*winv
            wmax = small.tile([128, NO], F16, tag="wmax")
            wmaxs = small.tile([128, NO], F32, tag="wmaxs")   # wmax/16129, f32
            wmaxh = small.tile([128, 2], F16, tag="wmaxh")    # w3 half-maxes
            wden = small.tile([128, NO], F32, tag="wden")
            winv = small.tile([128, NO], F32, tag="winv")
            bias2 = small.tile([128, NO], F32, tag="bias2")

            ps = [mmps.tile([128, TC], F32, tag="ps", name=f"ps{m}")
                  for m in range(NO)]
            st = [stps.tile([128, 1024], F16, tag="st", name=f"st{i}")
                  for i in range(2)]
            rowps = auxps.tile([128, 1024], F16, tag="rowps")  # row staging
            bcps = auxps.tile([128, TC], F32, tag="bcps")     # bcx broadcast

            # ---- constants ----
            nc.gpsimd.memset(ones1, 1.0)
            make_identity(nc, ident_f16)
            # preload the ACT activation table during the load phase so the
            # first real ACTIVATE doesn't eat the 1.3us ACT_TABLE_LOAD
            nc.scalar.activation(out=row_sb[0:1, 0:128], in_=ones1,
                                 func=ACTF.Copy, scale=1.0, bias=0.0)

            # ---- loads: x tile pairs first, w tiles after, w3 split ----
            for q in range(0, NT, 2):
                nc.sync.dma_start(out=x_sb[:, q:q + 2], in_=x3[:, q:q + 2])
            for r in range(NO - 1):
                nc.sync.dma_start(out=w_sb[:, r:r + 1], in_=w3[:, r:r + 1])
            nc.sync.dma_start(out=w_sb[:, 3, 0:512], in_=w3[:, 3, 0:512])
            nc.sync.dma_start(out=w_sb[:, 3, 512:WROW], in_=w3[:, 3, 512:WROW])

            # ---- PE warmup: HAM counts only real matmuls, so stream junk
            # N=128 MMs (into ps[3], cleared later by mm_group(3)'s
            # start=True) as soon as the first x tile lands — flips the PE
            # clock gate to 2.4GHz before the real transposes and MMs.
            for i in range(32):
                nc.tensor.matmul(ps[3][:, 0:128], lhsT=ident_f16,
                                 rhs=x_sb[:, 0, 0:128], start=True, stop=True)

            def x_red(q):
                nc.vector.tensor_reduce(
                    out=xmax[:, q:q + 1], in_=x_sb[:, q, :], axis=AX.X,
                    op=ALU.max, apply_absolute_value=True)
                nc.vector.tensor_scalar(
                    out=xden[:, q:q + 1], in0=xmax[:, q:q + 1],
                    scalar1=INV127, scalar2=EPS, op0=ALU.mult, op1=ALU.add)
                nc.vector.reciprocal(out=xinv[:, q:q + 1], in_=xden[:, q:q + 1])

            def x_quant_act(q, c0, c1):
                nc.scalar.activation(
                    out=xq[:, q, c0:c1], in_=x_sb[:, q, c0:c1], func=ACTF.Copy,
                    scale=xinv[:, q:q + 1], bias=MAGIC)

            def x_quant_pool(q, c0, c1):
                nc.gpsimd.tensor_scalar(
                    out=xq[:, q, c0:c1], in0=x_sb[:, q, c0:c1],
                    scalar1=xinv[:, q:q + 1], scalar2=MAGIC,
                    op0=ALU.mult, op1=ALU.add)

            def x_T(q):
                # all 8 contraction blocks of token-tile q into one st bank
                bank = st[q % 2]
                for k in range(KB):
                    nc.tensor.transpose(
                        bank[:, k * 128:(k + 1) * 128],
                        xq[:, q, k * 128:(k + 1) * 128], ident_f16)
                return bank

            def x_ev(q, bank, eng):
                # full-bank evict: st bank -> xqT[:, :, q-block], de-bias
                out_ap = xqT[:, 0:KB, q * 128:(q + 1) * 128]
                in_ap = bank.rearrange("p (k c) -> p k c", c=128)
                if eng == "act":
                    nc.scalar.activation(out=out_ap, in_=in_ap,
                                         func=ACTF.Copy, scale=1.0, bias=-MAGIC)
                else:
                    nc.vector.tensor_scalar(out=out_ap, in0=in_ap,
                                            scalar1=-MAGIC, scalar2=None,
                                            op0=ALU.add)

            def w_red(r):
                nc.vector.tensor_reduce(
                    out=wmax[:, r:r + 1], in_=w_sb[:, r, 0:IN_F], axis=AX.X,
                    op=ALU.max, apply_absolute_value=True)

            def w_den(r):
                nc.vector.tensor_scalar(
                    out=wden[:, r:r + 1], in0=wmax[:, r:r + 1],
                    scalar1=INV127, scalar2=EPS, op0=ALU.mult, op1=ALU.add)
                nc.vector.reciprocal(out=winv[:, r:r + 1], in_=wden[:, r:r + 1])
                nc.gpsimd.tensor_scalar(
                    out=wmaxs[:, r:r + 1], in0=wmax[:, r:r + 1],
                    scalar1=INV16129, scalar2=0.0,
                    op0=ALU.mult, op1=ALU.add)

            def pb_chain(r):
                # pbrow[o] = 127*b[o]*winv[o] as a row, staged via PE transpose
                nc.vector.tensor_tensor(
                    out=pb16[:, r:r + 1], in0=bias2[:, r:r + 1],
                    in1=winv[:, r:r + 1], op=ALU.mult)
                nc.tensor.transpose(
                    rowps[0:1, r * 128:(r + 1) * 128], pb16[:, r:r + 1],
                    ident_f16)
                nc.scalar.activation(
                    out=pbrow[0:1, r * 128:(r + 1) * 128],
                    in_=rowps[0:1, r * 128:(r + 1) * 128], func=ACTF.Copy,
                    scale=1.0, bias=0.0)

            def w_quant(r, c0, c1, eng):
                if eng == "act":
                    nc.scalar.activation(
                        out=wq[:, r, c0:c1], in_=w_sb[:, r, c0:c1],
                        func=ACTF.Copy, scale=winv[:, r:r + 1], bias=MAGIC)
                else:
                    nc.gpsimd.tensor_scalar(
                        out=wq[:, r, c0:c1], in0=w_sb[:, r, c0:c1],
                        scalar1=winv[:, r:r + 1], scalar2=MAGIC,
                        op0=ALU.mult, op1=ALU.add)

            def w_T(r):
                bank = st[r % 2]
                for k in range(KB):
                    nc.tensor.transpose(
                        bank[:, k * 128:(k + 1) * 128],
                        wq[:, r, k * 128:(k + 1) * 128], ident_f16)
                return bank

            def w_ev(r, bank, eng):
                out_ap = wqT[:, 0:KB, r * 128:(r + 1) * 128]
                in_ap = bank.rearrange("p (k c) -> p k c", c=128)
                if eng == "act":
                    nc.scalar.activation(out=out_ap, in_=in_ap,
                                         func=ACTF.Copy, scale=1.0, bias=-MAGIC)
                else:
                    nc.vector.tensor_scalar(out=out_ap, in0=in_ap,
                                            scalar1=-MAGIC, scalar2=None,
                                            op0=ALU.add)

            def mm_group(m):
                for k in range(KB):
                    nc.tensor.matmul(
                        ps[m], lhsT=wqT[:, k, m * 128:(m + 1) * 128],
                        rhs=xqT[:, k, :], start=(k == 0), stop=False)
                # bias as a rank-1 K=1 accumulation: contributes
                # 127*b*winv[o] * (xinv[t]/127) = b/(sw*sx) to ps, which the
                # out-pass scaling turns back into exactly +b.
                nc.tensor.matmul(
                    ps[m], lhsT=pbrow[0:1, m * 128:(m + 1) * 128],
                    rhs=xinvrow, start=False, stop=True)

            def out_evict(m):
                # u = ps * wmax/16129 (per-partition), fp16 out; PSUM on ACT
                u = obp.tile([128, TC], F16, tag="u", name=f"u{m}")
                nc.scalar.activation(out=u, in_=ps[m], func=ACTF.Copy,
                                     scale=wmaxs[:, m:m + 1], bias=0.0)
                # y = u * xmax[t]-broadcast (fp16 all-SBUF: DVE fast mode)
                osb = obp.tile([128, TC], F16, tag="osb", name=f"osb{m}")
                nc.vector.tensor_tensor(out=osb, in0=u, in1=bcx16,
                                        op=ALU.mult)
                nc.sync.dma_start(out=out_d[m * 128:(m + 1) * 128, :], in_=osb)

            # ================= pipelined emission (approx time order) ======
            # --- x chains ---
            x_red(0)
            x_quant_act(0, 0, 512)
            x_quant_pool(0, 512, IN_F)
            x_T(0)
            x_red(1)
            x_quant_act(1, 0, 512)
            x_quant_pool(1, 512, IN_F)
            x_ev(0, st[0], "act")
            x_T(1)
            x_red(2)
            x_quant_act(2, 0, 512)
            x_quant_pool(2, 512, IN_F)
            x_ev(1, st[1], "act")
            x_T(2)
            x_red(3)
            x_quant_act(3, 0, 512)
            x_quant_pool(3, 512, IN_F)
            x_ev(2, st[0], "act")
            x_T(3)
            # token-scale row: 4 single-column PE transposes into rowps
            for q in range(NT):
                nc.tensor.transpose(
                    rowps[0:1, q * 128:(q + 1) * 128], xmax[:, q:q + 1],
                    ident_f16)
            nc.scalar.activation(
                out=row_sb, in_=rowps[0:1, 0:512], func=ACTF.Copy,
                scale=1.0, bias=0.0)
            nc.tensor.matmul(bcps, lhsT=ones1, rhs=row_sb,
                             start=True, stop=True)
            # 1/(127*xmax) row for the bias rank-1 accumulation
            nc.vector.tensor_copy(out=xinv16, in_=xinv)
            for q in range(NT):
                nc.tensor.transpose(
                    rowps[0:1, 512 + q * 128:512 + (q + 1) * 128],
                    xinv16[:, q:q + 1], ident_f16)
            nc.scalar.activation(
                out=xinvrow, in_=rowps[0:1, 512:1024], func=ACTF.Copy,
                scale=INV127, bias=0.0)
            x_ev(3, st[1], "act")
            # bridge the PE idle gap before the w chains so HAM stays warm
            for i in range(12):
                nc.tensor.matmul(ps[3][:, 0:128], lhsT=ident_f16,
                                 rhs=x_sb[:, 0, 0:128], start=True, stop=True)

            # --- w chains ---
            w_red(0)
            w_den(0)
            w_quant(0, 0, IN_F, "pool")
            nc.gpsimd.tensor_copy(out=bias2[:, 0:1], in_=w_sb[:, 0, IN_F:WROW])
            w_T(0)
            w_red(1)
            w_den(1)
            w_ev(0, st[0], "act")
            w_quant(1, 0, IN_F, "pool")
            nc.gpsimd.tensor_copy(out=bias2[:, 1:2], in_=w_sb[:, 1, IN_F:WROW])
            w_T(1)
            nc.vector.tensor_copy(out=bcx16, in_=bcps)
            mm_group(0)
            w_red(2)
            w_den(2)
            w_ev(1, st[1], "act")
            w_quant(2, 0, IN_F, "pool")
            nc.gpsimd.tensor_copy(out=bias2[:, 2:3], in_=w_sb[:, 2, IN_F:WROW])
            w_T(2)
            mm_group(1)
            out_evict(0)
            # w3: reduce halves on DVE, quant ACT || Pool
            nc.vector.tensor_reduce(
                out=wmaxh[:, 0:1], in_=w_sb[:, 3, 0:512], axis=AX.X,
                op=ALU.max, apply_absolute_value=True)
            nc.vector.tensor_reduce(
                out=wmaxh[:, 1:2], in_=w_sb[:, 3, 512:IN_F], axis=AX.X,
                op=ALU.max, apply_absolute_value=True)
            nc.vector.tensor_tensor(
                out=wmax[:, 3:4], in0=wmaxh[:, 0:1], in1=wmaxh[:, 1:2],
                op=ALU.max)
            w_den(3)
            w_ev(2, st[0], "act")
            w_quant(3, 0, 512, "act")
            w_quant(3, 512, IN_F, "pool")
            nc.gpsimd.tensor_copy(out=bias2[:, 3:4], in_=w_sb[:, 3, IN_F:WROW])
            w_T(3)
            mm_group(2)
            out_evict(1)
            w_ev(3, st[1], "act")
            mm_group(3)
            out_evict(2)
            out_evict(3)

    _split_multiwaits(nc)
    return nc


def _split_multiwaits(nc):
    """Hoist all but one wait of any multi-wait instruction into standalone
    EventSemaphore instructions (the ISA carries one wait per instruction)."""
    import concourse.mybir as mybir

    fn = nc.m.functions[0]
    ctr = [0]
    for blk in fn.blocks:
        insts = list(blk.instructions)
        changed = False
        out = []
        for inst in insts:
            si = inst.sync_info
            waits = list(si.on_wait or []) if si is not None else []
            if len(waits) > 1:
                for w in waits[:-1]:
                    ctr[0] += 1
                    es = mybir.InstEventSemaphore(
                        name=f"I-eswait-{ctr[0]}", engine=inst.engine,
                        ins=[], outs=[],
                    )
                    es.sync_info = mybir.SyncInfo(on_wait=[w], on_update=[])
                    out.append(es)
                    nc.register_instruction(es)
                inst.sync_info = mybir.SyncInfo(
                    on_wait=[waits[-1]], on_update=list(si.on_update or []),
                )
                changed = True
            out.append(inst)
        if changed:
            blk.instructions = out


def get_nc():
    if "nc" not in _CACHE:
        _CACHE["nc"] = _build_nc()
    return _CACHE["nc"]


def make_in_maps(x, weight, bias):
    xf = np.asarray(x, dtype=np.float32).reshape(T, IN_F).astype(np.float16)
    w = np.asarray(weight, dtype=np.float32).astype(np.float16)
    b = np.asarray(bias, dtype=np.float32).astype(np.float16)
    wb = np.concatenate([w, b[:, None]], axis=1)
    in_maps = []
    for c in range(M_SHARDS * N_SHARDS):
        im, jn = divmod(c, N_SHARDS)
        in_maps.append({
            "x": np.ascontiguousarray(xf[im * TC:(im + 1) * TC]),
            "wb": np.ascontiguousarray(wb[jn * OC:(jn + 1) * OC]),
        })
    return in_maps


def assemble(results):
    y = np.empty((T, OUT_F), dtype=np.float32)
    for c in range(M_SHARDS * N_SHARDS):
        im, jn = divmod(c, N_SHARDS)
        y[im * TC:(im + 1) * TC, jn * OC:(jn + 1) * OC] = \
            results[c]["out"].T.astype(np.float32)
    return y.reshape(B, S, OUT_F)


def run(x, weight, bias, **spmd_kwargs):
    from concourse.bass_utils import run_bass_kernel_spmd

    nc = get_nc()
    in_maps = make_in_maps(x, weight, bias)
    res = run_bass_kernel_spmd(nc, in_maps, core_ids=list(range(8)), **spmd_kwargs)
    return assemble(res.results), res


def kernel(x, weight, bias):
    y, _ = run(x, weight, bias)
    return y


# revision 57
# speedup vs baseline: 1.0282x; 1.0271x over previous
"""CIM signed-magnitude linear kernel v4.4 — fp16 transport, fp16 stores.

Math identity (exact): y = (x_q @ w_q.T) * scale_x * scale_w.T + bias with
x_q = round(x / (max|x|/127 + eps)) per token, w_q likewise per out-channel.
(The reference's bit-serial/ADC path is numerically identical: per-chunk
partial sums are <= 64 so the [0,1023] ADC clamp never binds, and rounding
of integers is exact; int products accumulate exactly in f32 PSUM.)

Design (vs the f32 v2 baseline):
  * fp16 HBM transport for x / w / bias (host cast) — input DMA halves to
    ~2MB/core; fp16 output stores halve store traffic.  fp16 cast error
    ~2^-11/elem, small against the 2e-2 rel-err budget (measured 2.8e-3).
  * One-pass quantize: q_fp16 = (x * xinv) + 1536.0; the single f32->fp16
    rounding lands on the integer grid (fp16 ulp is exactly 1.0 on
    [1024, 2048)); the -1536 de-bias rides the mandatory PSUM->SBUF evict.
  * Per-operand cost is pass-count minimal: absmax reduce (DVE-only op) +
    quantize + transpose-evict.  The DVE reduce ladder (8x ~1.2us, no HW
    fast mode for reduce) is the critical resource; everything else is
    kept off DVE: quants on Pool/ACT, evicts on ACT in full-PSUM-bank
    [128,1024] ops, scale prep on Pool.
  * 2-hop out-pass: otmp_f32 = DVE TT(ps * xmax-broadcast) straight from
    PSUM (f32 intermediate — ps*xmax overflows fp16), then y = Pool
    TS(otmp * wmax/16129 + bias) using gpsimd's fast mult-AP+add-AP form
    (single-op add-AP falls onto a ~10x slower software path).
  * PE clock (HAM) management: 32 junk N=128 matmuls into ps[3] (cleared
    by mm_group(3)'s start=True) gated on the first x tile, plus a 12-MM
    bridge burst before the w chains — transposes never count as HAM
    activity, so only real MMs keep the PE at 2.4GHz.
  * bcx16 = broadcast of per-token xmax via PE column transposes + ones
    matmul.  Beware PSUM bank collisions: a PE write and an engine read to
    different addresses of the SAME bank is a HW fault, so each staging
    row/broadcast gets its own bank region ordering via overlapping APs.

Sharding: 8 cores = 4 token-shards x 2 out-feature shards, no collectives.
"""

import os

os.environ.setdefault("JAX_PLATFORMS", "cpu")

import numpy as np

B, S, IN_F, OUT_F = 2, 1024, 1024, 1024
T = B * S
M_SHARDS, N_SHARDS = 4, 2
TC = T // M_SHARDS             # 512 tokens per core
OC = OUT_F // N_SHARDS         # 512 out-features per core
NT = TC // 128                 # 4 token tiles
NO = OC // 128                 # 4 out-feature tiles
KB = IN_F // 128               # 8 contraction blocks
WROW = IN_F + 1                # weight row with bias appended

EPS = 1e-8
INV127 = 1.0 / 127.0
INV16129 = 1.0 / 16129.0
# fp16 has 10 mantissa bits: on [1024,2048) the ulp is exactly 1.0, so the
# single f32->fp16 output rounding of (q + 1536) lands on the integer grid.
MAGIC = 1536.0
N_WARMUP = 0

_CACHE = {}


def _build_nc():
    import concourse.bass as bass
    import concourse.mybir as mybir
    import concourse.tile as tile
    from concourse.masks import make_identity

    F32 = mybir.dt.float32
    F16 = mybir.dt.float16
    ALU = mybir.AluOpType
    ACTF = mybir.ActivationFunctionType
    AX = mybir.AxisListType

    nc = bass.Bass("TRN2", target_bir_lowering=False, debug=False)

    x_d = nc.dram_tensor("x", [TC, IN_F], F16, kind="ExternalInput").ap()
    wb_d = nc.dram_tensor("wb", [OC, WROW], F16, kind="ExternalInput").ap()
    out_d = nc.dram_tensor("out", [OC, TC], F16, kind="ExternalOutput").ap()

    x3 = x_d.rearrange("(q p) i -> p q i", p=128)     # [128, NT, IN_F]
    w3 = wb_d.rearrange("(r p) i -> p r i", p=128)    # [128, NO, WROW]

    with tile.TileContext(nc) as tc:
        with (
            tc.tile_pool(name="raw", bufs=1) as raw,
            tc.tile_pool(name="qb", bufs=1) as qb,
            tc.tile_pool(name="small", bufs=1) as small,
            tc.tile_pool(name="ob", bufs=2) as obp,
            tc.tile_pool(name="mmps", bufs=4, space="PSUM") as mmps,
            tc.tile_pool(name="stps", bufs=2, space="PSUM") as stps,
            tc.tile_pool(name="auxps", bufs=1, space="PSUM") as auxps,
        ):
            x_sb = raw.tile([128, NT, IN_F], F16, tag="x_sb")
            w_sb = raw.tile([128, NO, WROW], F16, tag="w_sb")
            xq = qb.tile([128, NT, IN_F], F16, tag="xq")      # values +1536
            wq = qb.tile([128, NO, IN_F], F16, tag="wq")      # values +1536
            xqT = qb.tile([128, KB, TC], F16, tag="xqT")      # true ints
            wqT = qb.tile([128, KB, OC], F16, tag="wqT")      # true ints
            bcx16 = qb.tile([128, TC], F16, tag="bcx16")      # xmax broadcast
            row_sb = qb.tile([1, TC], F16, tag="row_sb")
            xinvrow = qb.tile([1, TC], F16, tag="xinvrow")    # (1/xmax)/127 row
            pbrow = qb.tile([1, OC], F16, tag="pbrow")        # 127*b*winv row
            ones1 = qb.tile([1, 128], F16, tag="ones1")
            ident_f16 = qb.tile([128, 128], F16, tag="ident_f16")

            # fp16 reduce outputs: max|fp16| is exactly representable in
            # fp16, and an all-2-byte instruction lets the DVE engage its
            # 2x/4x perf modes.
            xmax = small.tile([128, NT], F16, tag="xmax")
            xden = small.tile([128, NT], F32, tag="xden")
            xinv = small.tile([128, NT], F32, tag="xinv")
            xinv16 = small.tile([128, NT], F16, tag="xinv16")  # xinv/127
            pb16 = small.tile([128, NO], F16, tag="pb16")      # 127*b*winv
            wmax = small.tile([128, NO], F16, tag="wmax")
            wmaxs = small.tile([128, NO], F32, tag="wmaxs")   # wmax/16129, f32
            wmaxh = small.tile([128, 2], F16, tag="wmaxh")    # w3 half-maxes
            wden = small.tile([128, NO], F32, tag="wden")
            winv = small.tile([128, NO], F32, tag="winv")
            bias2 = small.tile([128, NO], F32, tag="bias2")

            ps = [mmps.tile([128, TC], F32, tag="ps", name=f"ps{m}")
                  for m in range(NO)]
            st = [stps.tile([128, 1024], F16, tag="st", name=f"st{i}")
                  for i in range(2)]
            rowps = auxps.tile([128, 1024], F16, tag="rowps")  # row staging
            bcps = auxps.tile([128, TC], F32, tag="bcps")     # bcx broadcast

            # ---- constants ----
            nc.gpsimd.memset(ones1, 1.0)
            make_identity(nc, ident_f16)
            # preload the ACT activation table during the load phase so the
            # first real ACTIVATE doesn't eat the 1.3us ACT_TABLE_LOAD
            nc.scalar.activation(out=row_sb[0:1, 0:128], in_=ones1,
                                 func=ACTF.Copy, scale=1.0, bias=0.0)

            # ---- loads: x tile pairs first, w tiles after, w3 split ----
            for q in range(0, NT, 2):
                nc.sync.dma_start(out=x_sb[:, q:q + 2], in_=x3[:, q:q + 2])
            for r in range(NO - 1):
                nc.sync.dma_start(out=w_sb[:, r:r + 1], in_=w3[:, r:r + 1])
            nc.sync.dma_start(out=w_sb[:, 3, 0:512], in_=w3[:, 3, 0:512])
            nc.sync.dma_start(out=w_sb[:, 3, 512:WROW], in_=w3[:, 3, 512:WROW])

            # ---- PE warmup: HAM counts only real matmuls, so stream junk
            # N=128 MMs (into ps[3], cleared later by mm_group(3)'s
            # start=True) as soon as the first x tile lands — flips the PE
            # clock gate to 2.4GHz before the real transposes and MMs.
            for i in range(32):
                nc.tensor.matmul(ps[3][:, 0:128], lhsT=ident_f16,
                                 rhs=x_sb[:, 0, 0:128], start=True, stop=True)

            def x_red(q):
                nc.vector.tensor_reduce(
                    out=xmax[:, q:q + 1], in_=x_sb[:, q, :], axis=AX.X,
                    op=ALU.max, apply_absolute_value=True)
                nc.vector.tensor_scalar(
                    out=xden[:, q:q + 1], in0=xmax[:, q:q + 1],
                    scalar1=INV127, scalar2=EPS, op0=ALU.mult, op1=ALU.add)
                nc.vector.reciprocal(out=xinv[:, q:q + 1], in_=xden[:, q:q + 1])

            def x_quant_act(q, c0, c1):
                nc.scalar.activation(
                    out=xq[:, q, c0:c1], in_=x_sb[:, q, c0:c1], func=ACTF.Copy,
                    scale=xinv[:, q:q + 1], bias=MAGIC)

            def x_quant_pool(q, c0, c1):
                nc.gpsimd.tensor_scalar(
                    out=xq[:, q, c0:c1], in0=x_sb[:, q, c0:c1],
                    scalar1=xinv[:, q:q + 1], scalar2=MAGIC,
                    op0=ALU.mult, op1=ALU.add)

            def x_T(q):
                # all 8 contraction blocks of token-tile q into one st bank
                bank = st[q % 2]
                for k in range(KB):
                    nc.tensor.transpose(
                        bank[:, k * 128:(k + 1) * 128],
                        xq[:, q, k * 128:(k + 1) * 128], ident_f16)
                return bank

            def x_ev(q, bank, eng):
                # full-bank evict: st bank -> xqT[:, :, q-block], de-bias
                out_ap = xqT[:, 0:KB, q * 128:(q + 1) * 128]
                in_ap = bank.rearrange("p (k c) -> p k c", c=128)
                if eng == "act":
                    nc.scalar.activation(out=out_ap, in_=in_ap,
                                         func=ACTF.Copy, scale=1.0, bias=-MAGIC)
                else:
                    nc.vector.tensor_scalar(out=out_ap, in0=in_ap,
                                            scalar1=-MAGIC, scalar2=None,
                                            op0=ALU.add)

            def w_red(r):
                nc.vector.tensor_reduce(
                    out=wmax[:, r:r + 1], in_=w_sb[:, r, 0:IN_F], axis=AX.X,
                    op=ALU.max, apply_absolute_value=True)

            def w_den(r):
                nc.vector.tensor_scalar(
                    out=wden[:, r:r + 1], in0=wmax[:, r:r + 1],
                    scalar1=INV127, scalar2=EPS, op0=ALU.mult, op1=ALU.add)
                nc.vector.reciprocal(out=winv[:, r:r + 1], in_=wden[:, r:r + 1])
                nc.gpsimd.tensor_scalar(
                    out=wmaxs[:, r:r + 1], in0=wmax[:, r:r + 1],
                    scalar1=INV16129, scalar2=0.0,
                    op0=ALU.mult, op1=ALU.add)

            def pb_chain(r):
                # pbrow[o] = 127*b[o]*winv[o] as a row, staged via PE transpose
                nc.vector.tensor_tensor(
                    out=pb16[:, r:r + 1], in0=bias2[:, r:r + 1],
                    in1=winv[:, r:r + 1], op=ALU.mult)
                nc.tensor.transpose(
                    rowps[0:1, r * 128:(r + 1) * 128], pb16[:, r:r + 1],
                    ident_f16)
                nc.scalar.activation(
                    out=pbrow[0:1, r * 128:(r + 1) * 128],
                    in_=rowps[0:1, r * 128:(r + 1) * 128], func=ACTF.Copy,
                    scale=1.0, bias=0.0)

            def w_quant(r, c0, c1, eng):
                if eng == "act":
                    nc.scalar.activation(
                        out=wq[:, r, c0:c1], in_=w_sb[:, r, c0:c1],
                        func=ACTF.Copy, scale=winv[:, r:r + 1], bias=MAGIC)
                else:
                    nc.gpsimd.tensor_scalar(
                        out=wq[:, r, c0:c1], in0=w_sb[:, r, c0:c1],
                        scalar1=winv[:, r:r + 1], scalar2=MAGIC,
                        op0=ALU.mult, op1=ALU.add)

            def w_T(r):
                bank = st[r % 2]
                for k in range(KB):
                    nc.tensor.transpose(
                        bank[:, k * 128:(k + 1) * 128],
                        wq[:, r, k * 128:(k + 1) * 128], ident_f16)
                return bank

            def w_ev(r, bank, eng):
                out_ap = wqT[:, 0:KB, r * 128:(r + 1) * 128]
                in_ap = bank.rearrange("p (k c) -> p k c", c=128)
                if eng == "act":
                    nc.scalar.activation(out=out_ap, in_=in_ap,
                                         func=ACTF.Copy, scale=1.0, bias=-MAGIC)
                else:
                    nc.vector.tensor_scalar(out=out_ap, in0=in_ap,
                                            scalar1=-MAGIC, scalar2=None,
                                            op0=ALU.add)

            def mm_group(m):
                for k in range(KB):
                    nc.tensor.matmul(
                        ps[m], lhsT=wqT[:, k, m * 128:(m + 1) * 128],
                        rhs=xqT[:, k, :], start=(k == 0), stop=False)
                # bias as a rank-1 K=1 accumulation: contributes
                # 127*b*winv[o] * (xinv[t]/127) = b/(sw*sx) to ps, which the
                # out-pass scaling turns back into exactly +b.
                nc.tensor.matmul(
                    ps[m], lhsT=pbrow[0:1, m * 128:(m + 1) * 128],
                    rhs=xinvrow, start=False, stop=True)

            def out_evict(m):
                # u = ps * wmax/16129 (per-partition), fp16 out; PSUM on ACT
                u = obp.tile([128, TC], F16, tag="u", name=f"u{m}")
                nc.scalar.activation(out=u, in_=ps[m], func=ACTF.Copy,
                                     scale=wmaxs[:, m:m + 1], bias=0.0)
                # y = u * xmax[t]-broadcast (fp16 all-SBUF: DVE fast mode)
                osb = obp.tile([128, TC], F16, tag="osb", name=f"osb{m}")
                nc.vector.tensor_tensor(out=osb, in0=u, in1=bcx16,
                                        op=ALU.mult)
                nc.sync.dma_start(out=out_d[m * 128:(m + 1) * 128, :], in_=osb)

            # ================= pipelined emission (approx time order) ======
            # --- x chains ---
            x_red(0)
            x_quant_act(0, 0, 512)
            x_quant_pool(0, 512, IN_F)
            x_T(0)
            x_red(1)
            x_quant_act(1, 0, 512)
            x_quant_pool(1, 512, IN_F)
            x_ev(0, st[0], "act")
            x_T(1)
            x_red(2)
            x_quant_act(2, 0, 512)
            x_quant_pool(2, 512, IN_F)
            x_ev(1, st[1], "act")
            x_T(2)
            x_red(3)
            x_quant_act(3, 0, 512)
            x_quant_pool(3, 512, IN_F)
            x_ev(2, st[0], "act")
            x_T(3)
            # token-scale row: 4 single-column PE transposes into rowps
            for q in range(NT):
                nc.tensor.transpose(
                    rowps[0:1, q * 128:(q + 1) * 128], xmax[:, q:q + 1],
                    ident_f16)
            nc.scalar.activation(
                out=row_sb, in_=rowps[0:1, 0:512], func=ACTF.Copy,
                scale=1.0, bias=0.0)
            nc.tensor.matmul(bcps, lhsT=ones1, rhs=row_sb,
                             start=True, stop=True)
            # 1/(127*xmax) row for the bias rank-1 accumulation
            nc.vector.tensor_copy(out=xinv16, in_=xinv)
            for q in range(NT):
                nc.tensor.transpose(
                    rowps[0:1, 512 + q * 128:512 + (q + 1) * 128],
                    xinv16[:, q:q + 1], ident_f16)
            nc.scalar.activation(
                out=xinvrow, in_=rowps[0:1, 512:1024], func=ACTF.Copy,
                scale=INV127, bias=0.0)
            x_ev(3, st[1], "act")
            # bridge the PE idle gap before the w chains so HAM stays warm
            for i in range(12):
                nc.tensor.matmul(ps[3][:, 0:128], lhsT=ident_f16,
                                 rhs=x_sb[:, 0, 0:128], start=True, stop=True)

            # --- w chains ---
            w_red(0)
            w_den(0)
            w_quant(0, 0, IN_F, "pool")
            nc.gpsimd.tensor_copy(out=bias2[:, 0:1], in_=w_sb[:, 0, IN_F:WROW])
            w_T(0)
            w_red(1)
            w_den(1)
            w_ev(0, st[0], "act")
            w_quant(1, 0, IN_F, "pool")
            nc.gpsimd.tensor_copy(out=bias2[:, 1:2], in_=w_sb[:, 1, IN_F:WROW])
            w_T(1)
            nc.vector.tensor_copy(out=bcx16, in_=bcps)
            mm_group(0)
            w_red(2)
            w_den(2)
            w_ev(1, st[1], "dve")
            w_quant(2, 0, IN_F, "pool")
            nc.gpsimd.tensor_copy(out=bias2[:, 2:3], in_=w_sb[:, 2, IN_F:WROW])
            w_T(2)
            mm_group(1)
            out_evict(0)
            # w3: reduce halves on DVE, quant ACT || Pool
            nc.vector.tensor_reduce(
                out=wmaxh[:, 0:1], in_=w_sb[:, 3, 0:512], axis=AX.X,
                op=ALU.max, apply_absolute_value=True)
            nc.vector.tensor_reduce(
                out=wmaxh[:, 1:2], in_=w_sb[:, 3, 512:IN_F], axis=AX.X,
                op=ALU.max, apply_absolute_value=True)
            nc.vector.tensor_tensor(
                out=wmax[:, 3:4], in0=wmaxh[:, 0:1], in1=wmaxh[:, 1:2],
                op=ALU.max)
            w_den(3)
            w_ev(2, st[0], "act")
            w_quant(3, 0, 512, "act")
            w_quant(3, 512, IN_F, "pool")
            nc.gpsimd.tensor_copy(out=bias2[:, 3:4], in_=w_sb[:, 3, IN_F:WROW])
            w_T(3)
            mm_group(2)
            out_evict(1)
            w_ev(3, st[1], "act")
            mm_group(3)
            out_evict(2)
            out_evict(3)

    _split_multiwaits(nc)
    return nc


def _split_multiwaits(nc):
    """Hoist all but one wait of any multi-wait instruction into standalone
    EventSemaphore instructions (the ISA carries one wait per instruction)."""
    import concourse.mybir as mybir

    fn = nc.m.functions[0]
    ctr = [0]
    for blk in fn.blocks:
        insts = list(blk.instructions)
        changed = False
        out = []
        for inst in insts:
            si = inst.sync_info
            waits = list(si.on_wait or []) if si is not None else []
            if len(waits) > 1:
                for w in waits[:-1]:
                    ctr[0] += 1
                    es = mybir.InstEventSemaphore(
                        name=f"I-eswait-{ctr[0]}", engine=inst.engine,
                        ins=[], outs=[],
                    )
                    es.sync_info = mybir.SyncInfo(on_wait=[w], on_update=[])
                    out.append(es)
                    nc.register_instruction(es)
                inst.sync_info = mybir.SyncInfo(
                    on_wait=[waits[-1]], on_update=list(si.on_update or []),
                )
                changed = True
            out.append(inst)
        if changed:
            blk.instructions = out


def get_nc():
    if "nc" not in _CACHE:
        _CACHE["nc"] = _build_nc()
    return _CACHE["nc"]


def make_in_maps(x, weight, bias):
    xf = np.asarray(x, dtype=np.float32).reshape(T, IN_F).astype(np.float16)
    w = np.asarray(weight, dtype=np.float32).astype(np.float16)
    b = np.asarray(bias, dtype=np.float32).astype(np.float16)
    wb = np.concatenate([w, b[:, None]], axis=1)
    in_maps = []
    for c in range(M_SHARDS * N_SHARDS):
        im, jn = divmod(c, N_SHARDS)
        in_maps.append({
            "x": np.ascontiguousarray(xf[im * TC:(im + 1) * TC]),
            "wb": np.ascontiguousarray(wb[jn * OC:(jn + 1) * OC]),
        })
    return in_maps


def assemble(results):
    y = np.empty((T, OUT_F), dtype=np.float32)
    for c in range(M_SHARDS * N_SHARDS):
        im, jn = divmod(c, N_SHARDS)
        y[im * TC:(im + 1) * TC, jn * OC:(jn + 1) * OC] = \
            results[c]["out"].T.astype(np.float32)
    return y.reshape(B, S, OUT_F)


def run(x, weight, bias, **spmd_kwargs):
    from concourse.bass_utils import run_bass_kernel_spmd

    nc = get_nc()
    in_maps = make_in_maps(x, weight, bias)
    res = run_bass_kernel_spmd(nc, in_maps, core_ids=list(range(8)), **spmd_kwargs)
    return assemble(res.results), res


def kernel(x, weight, bias):
    y, _ = run(x, weight, bias)
    return y
